# revision 1
# baseline (speedup 1.0000x reference)
"""Trainium2 Bass kernel for prefix-KV causal attention (nn_Attn_38757784879167).

Sharding: 8 cores <- (b, h) pairs (B=2 x H=4). Each core runs the full
attention for one (batch, head): QKV projection, S^T-layout flash attention
(scores computed transposed: keys on partitions, queries on free dim -> no
P transposes needed), PV + rowsum fused via an appended ones-row on V,
normalization + per-head out-projection partial. Host sums the 4 per-head
partials per batch (the out-projection "all-reduce" gather step).

Layout core ideas:
  - All per-core inputs are host-packed into ONE [128, NW] f32 tensor ->
    a single input DMA (fp32 matmuls can carry only one sync wait, so
    downstream waits must collapse onto one DMA lane).
  - S^T tile [128 keys, 512 queries] = matmul(lhsT=kT[:,chunk] [32,128],
    rhs=qT[:,qtile] [32,512]) in fp32r (1 cyc/row at N>=256).
  - exp on ScalarE in 3-chunk batches ([128,1536] PSUM->SBUF) to amortize
    per-instruction overhead; softmax max-subtraction is skipped (scores
    *1/sqrt(hd) are ~N(0,1), global max ~9.5 -> exp is fp32-safe).
  - v' = [v | 1] ([128,33] chunks) makes one PV matmul produce both
    ctx^T (rows 0..31) and the softmax row-sums (row 32).
  - shifted-causal mask applied as a 0/1 multiply on the 4 diagonal chunks
    of each query tile (mask tiles precomputed on host).
"""

import math
import os

import numpy as np

B = 2
T = 4096
D = 128
H = 4
HD = 32
PRE = 2048
CH = 128  # keys per chunk (partition dim of S^T tiles)
QT = 512  # queries per tile (free dim of S^T tiles)
GRP = 3  # chunks per exp batch (3 PSUM banks)

_CACHE = {}


def _offsets(T, PRE):
    """Column offsets into the packed fp16 [128, NW] input tensor."""
    diag = QT // CH
    nch = (T + PRE) // CH
    o = {}
    o["x"] = 0  # xT [128, T] fp16
    o["pk"] = T  # pkT image [128, PRE] fp16, rows 32..127 zero
    o["pv"] = T + PRE  # full vS image [128, nch*(HD+1)] fp16
    o["m"] = o["pv"] + nch * (HD + 2)  # mask [128, diag*QT] fp16
    o["wq"] = o["m"] + diag * QT
    o["wk"] = o["wq"] + HD
    o["wv"] = o["wk"] + HD
    o["wo"] = o["wv"] + HD  # wout rows 0..31 [*, D] fp16
    o["nw"] = o["wo"] + D
    return o


def build_attn(T=T, PRE=PRE, grp=GRP):
    """Build + compile the per-core Bacc module. Returns nc."""
    from contextlib import ExitStack

    import concourse.mybir as mybir
    import concourse.tile as tile
    from concourse import bacc

    f32 = mybir.dt.float32
    fp16 = mybir.dt.float16
    EXP = mybir.ActivationFunctionType.Exp
    TK = PRE + T
    NCH = TK // CH
    NQT = T // QT
    DIAG = QT // CH
    SCALE = 1.0 / math.sqrt(HD)
    O = _offsets(T, PRE)

    nc = bacc.Bacc("TRN2", target_bir_lowering=False, debug=False)

    pack_d = nc.dram_tensor("pack", [128, O["nw"]], fp16, kind="ExternalInput")
    out_d = nc.dram_tensor("out", [T, D], f32, kind="ExternalOutput")
    dbg = bool(int(os.environ.get("ATTN_DBG", "0")))
    if dbg:
        dbgq_d = nc.dram_tensor("dbgq", [128, T], fp16, kind="ExternalOutput")
        dbgk_d = nc.dram_tensor("dbgk", [128, T], fp16, kind="ExternalOutput")
        dbgm_d = nc.dram_tensor("dbgm", [128, O["nw"] - O["pv"]], fp16, kind="ExternalOutput")
        dbgc_d = nc.dram_tensor("dbgc", [T // QT, HD, QT], fp16, kind="ExternalOutput")
        dbgr_d = nc.dram_tensor("dbgr", [T // QT, 1, QT], f32, kind="ExternalOutput")

    with tile.TileContext(nc) as tc, ExitStack() as ctx:
        const = ctx.enter_context(tc.tile_pool(name="const", bufs=1))
        packed = const.tile([128, O["nw"]], fp16, tag="pack")
        qT_s = const.tile([128, T], fp16, tag="qT")
        kT_s = const.tile([128, T], fp16, tag="kT")  # projected keys only
        vS_s = const.tile([128, (T // CH) * (HD + 2)], fp16, tag="vS")  # new-v
        ones_s = const.tile([1, 1], f32, tag="ones")
        nb_s = const.tile([128, 1], f32, tag="nb")

        nc.sync.dma_start(packed[:, 0 : O["pk"]], pack_d[:, 0 : O["pk"]])
        nc.sync.dma_start(packed[:, O["pk"] :], pack_d[:, O["pk"] :])

        xT = packed[:, O["x"] : O["x"] + T]
        wq = packed[:, O["wq"] : O["wq"] + HD]
        wk = packed[:, O["wk"] : O["wk"] + HD]
        wv = packed[:, O["wv"] : O["wv"] + HD]
        wout = packed[0:HD, O["wo"] : O["wo"] + D]
        mask_s = packed[:, O["m"] : O["m"] + DIAG * QT]

        nc.vector.memset(ones_s[:], 1.0)
        nc.vector.memset(nb_s[:], -5.0)
        nc.vector.memset(qT_s[32:64, :], 0.0)
        nc.vector.memset(qT_s[64:128, :], 0.0)
        nc.vector.memset(kT_s[32:64, :], 0.0)
        nc.vector.memset(kT_s[64:128, :], 0.0)

        vS_3d = vS_s[:].rearrange("p (c e) -> p c e", e=HD + 2)
        nc.vector.memset(vS_3d[:, :, HD], 1.0)
        nc.vector.memset(vS_3d[:, :, HD + 1], 0.0)

        # ---- preamble: q/k/v projections (fp16 in, fp32 accumulate),
        # interleaved per 512-tile so qtile 0 attention can start early ----
        with tc.tile_pool(name="prePS", bufs=4, space="PSUM") as pre_ps:
            for t0 in range(0, T, QT):
                psq = pre_ps.tile([HD, QT], f32, tag="p")
                nc.tensor.matmul(psq[:], wq, xT[:, t0 : t0 + QT])
                nc.vector.tensor_copy(qT_s[0:HD, t0 : t0 + QT], psq[:])
                psk = pre_ps.tile([HD, QT], f32, tag="p")
                nc.tensor.matmul(psk[:], wk, xT[:, t0 : t0 + QT])
                nc.vector.tensor_copy(kT_s[0:HD, t0 : t0 + QT], psk[:])
                for i in range(t0 // CH, (t0 + QT) // CH):
                    psv = pre_ps.tile([CH, HD], f32, tag="p")
                    nc.tensor.matmul(psv[:], xT[:, CH * i : CH * (i + 1)], wv)
                    nc.vector.tensor_copy(vS_3d[:, i, 0:HD], psv[:])

        # ---- attention ----
        psS = ctx.enter_context(tc.tile_pool(name="psS", bufs=2, space="PSUM"))
        psCE = ctx.enter_context(tc.tile_pool(name="psCE", bufs=1, space="PSUM"))
        psOP = ctx.enter_context(tc.tile_pool(name="psOP", bufs=1, space="PSUM"))
        ptp = ctx.enter_context(tc.tile_pool(name="pt", bufs=9))
        epp = ctx.enter_context(tc.tile_pool(name="ep", bufs=3))
        outp = ctx.enter_context(tc.tile_pool(name="outp", bufs=4))

        NPRE = PRE // CH

        def kT_chunk(c):
            if c < NPRE:
                return packed[:, O["pk"] + CH * c : O["pk"] + CH * (c + 1)]
            return kT_s[:, CH * (c - NPRE) : CH * (c - NPRE + 1)]

        pvimg = packed[:, O["pv"] : O["m"]].rearrange("p (c e) -> p c e", e=HD + 2)

        def v_chunk(c):
            if c < NPRE:
                return pvimg[:, c, :]
            return vS_3d[:, c - NPRE, :]

        for j in range(NQT):
            nch = (PRE + QT * (j + 1)) // CH
            psCfull = psCE.tile([128, QT], f32, tag="ce")
            psC = psCfull[0 : HD + 2, :]
            for c0 in range(0, nch, grp):
                c1 = min(c0 + grp, nch)
                w = (c1 - c0) * QT
                ps = psS.tile([CH, grp * QT], f32, tag="s")
                with tc.high_priority(offset=10):
                    for c in range(c0, c1):
                        off = (c - c0) * QT
                        nc.tensor.matmul(
                            ps[:, off : off + QT],
                            kT_chunk(c),
                            qT_s[:, QT * j : QT * (j + 1)],
                        )
                pt = ptp.tile([CH, grp * QT], fp16, tag="pt")
                nc.scalar.activation(pt[:, 0:w], ps[:, 0:w], EXP, scale=SCALE, bias=nb_s[:])
                for c in range(c0, c1):
                    off = (c - c0) * QT
                    d = c - (nch - DIAG)
                    if d >= 0:
                        nc.vector.tensor_mul(
                            pt[:, off : off + QT],
                            pt[:, off : off + QT],
                            mask_s[:, QT * d : QT * (d + 1)],
                        )
                    nc.tensor.matmul(
                        psC,
                        v_chunk(c),
                        pt[:, off : off + QT],
                        start=(c == 0),
                        stop=(c == nch - 1),
                        skip_group_check=True,
                    )

            # ---- epilogue for this query tile ----
            ctxT_s = epp.tile([HD, QT], fp16, tag="ctxT")
            nc.vector.tensor_copy(ctxT_s[:], psC[0:HD, :])
            rs_s = epp.tile([1, QT], f32, tag="rs")
            nc.vector.tensor_copy(rs_s[:], psC[HD : HD + 1, :])
            if dbg:
                nc.sync.dma_start(dbgc_d[j], ctxT_s[:])
                nc.sync.dma_start(dbgr_d[j], rs_s[:])
            psR = psCE.tile([128, QT // 128], f32, tag="ce")
            for jj in range(QT // 128):
                nc.tensor.matmul(
                    psR[:, jj : jj + 1],
                    rs_s[0:1, 128 * jj : 128 * (jj + 1)],
                    ones_s[:],
                )
            rsT_s = epp.tile([128, QT // 128], f32, tag="rsT")
            nc.vector.tensor_copy(rsT_s[:], psR[:])
            rec_s = epp.tile([128, QT // 128], f32, tag="rec")
            nc.vector.reciprocal(rec_s[:], rsT_s[:])
            psO = psOP.tile([128, (QT // 128) * D], f32, tag="o")
            ot = outp.tile([128, (QT // 128) * D], f32, tag="o")
            for jj in range(QT // 128):
                nc.tensor.matmul(
                    psO[:, D * jj : D * (jj + 1)],
                    ctxT_s[:, 128 * jj : 128 * (jj + 1)],
                    wout,
                )
                nc.vector.tensor_scalar_mul(
                    ot[:, D * jj : D * (jj + 1)],
                    psO[:, D * jj : D * (jj + 1)],
                    rec_s[:, jj : jj + 1],
                )
            for jj in range(QT // 128):
                r0 = QT * j + 128 * jj
                nc.sync.dma_start(
                    out_d[r0 : r0 + 128, :], ot[:, D * jj : D * (jj + 1)]
                )

        if dbg:
            nc.sync.dma_start(dbgq_d[:], qT_s[:])
            nc.sync.dma_start(dbgk_d[:], kT_s[:])
            nc.sync.dma_start(dbgm_d[:], packed[:, O["pv"] :])

    nc.compile()
    return nc


def _make_masks(qt=QT, ch=CH):
    """Multiplicative mask: 1 where attending allowed, 0 where disallowed."""
    diag = qt // ch
    m = np.zeros((ch, diag * qt), dtype=np.float32)
    p = np.arange(ch)[:, None]
    t = np.arange(qt)[None, :]
    for d in range(diag):
        m[:, qt * d : qt * (d + 1)] = (t >= ch * d + p).astype(np.float32)
    return m


def pack_inputs(x_b, pk_bh, pv_bh, wq, wk, wv, wout_h, Tv=T, PREv=PRE):
    """Pack one core's inputs into the [128, NW] fp16 tensor."""
    O = _offsets(Tv, PREv)
    p = np.zeros((128, O["nw"]), dtype=np.float16)
    p[:, O["x"] : O["x"] + Tv] = x_b.T
    p[0:HD, O["pk"] : O["pk"] + PREv] = pk_bh.T
    nch = (Tv + PREv) // CH
    vimg = np.zeros((128, nch, HD + 2), dtype=np.float16)
    vimg[:, :, HD] = 1.0
    vimg[:, 0 : PREv // CH, 0:HD] = pv_bh.reshape(PREv // CH, CH, HD).transpose(
        1, 0, 2
    )
    p[:, O["pv"] : O["m"]] = vimg.reshape(128, -1)
    p[:, O["m"] : O["m"] + (QT // CH) * QT] = _make_masks()
    p[:, O["wq"] : O["wq"] + HD] = wq
    p[:, O["wk"] : O["wk"] + HD] = wk
    p[:, O["wv"] : O["wv"] + HD] = wv
    p[0:HD, O["wo"] : O["wo"] + D] = wout_h
    return p


def make_in_maps(x, pk, pv, Wqkv, Wout):
    in_maps = []
    for b in range(B):
        for h in range(H):
            in_maps.append(
                {
                    "pack": pack_inputs(
                        np.asarray(x[b], dtype=np.float32),
                        np.asarray(pk[b, h], dtype=np.float32),
                        np.asarray(pv[b, h], dtype=np.float32),
                        np.asarray(Wqkv[:, h * HD : (h + 1) * HD], dtype=np.float32),
                        np.asarray(
                            Wqkv[:, D + h * HD : D + (h + 1) * HD], dtype=np.float32
                        ),
                        np.asarray(
                            Wqkv[:, 2 * D + h * HD : 2 * D + (h + 1) * HD],
                            dtype=np.float32,
                        ),
                        np.asarray(Wout[h * HD : (h + 1) * HD, :], dtype=np.float32),
                    )
                }
            )
    return in_maps


def _install_ntff_shim():
    """Provide antenv.axon_hooks (absent in this image) so trace=True works.

    Replicates trn_boot._ntff_profile_via_ctypes against /opt/axon/libaxon_pjrt.so.
    """
    import contextlib
    import ctypes
    import sys
    import types

    try:
        from antenv.axon_hooks import get_axon_ntff_profile_hook  # noqa: F401

        return True
    except ImportError:
        pass
    so_path = "/opt/axon/libaxon_pjrt.so"
    if not os.path.exists(so_path):
        return False
    lib = ctypes.CDLL(so_path)
    if not hasattr(lib, "axon_start_nrt_profile"):
        return False
    lib.axon_start_nrt_profile.argtypes = [
        ctypes.POINTER(ctypes.c_int64),
        ctypes.c_size_t,
    ]
    lib.axon_start_nrt_profile.restype = ctypes.c_int64
    lib.axon_stop_nrt_profile.argtypes = [ctypes.c_char_p]
    lib.axon_stop_nrt_profile.restype = ctypes.c_int64

    @contextlib.contextmanager
    def _hook(output_dir, device_ids):
        import jax

        jax.devices()
        if device_ids:
            ids = (ctypes.c_int64 * len(device_ids))(*device_ids)
            rc = lib.axon_start_nrt_profile(ids, len(device_ids))
        else:
            rc = lib.axon_start_nrt_profile(None, 0)
        if rc != 0:
            raise RuntimeError(f"axon_start_nrt_profile rc={rc}")
        try:
            yield
        finally:
            n = lib.axon_stop_nrt_profile(str(output_dir).encode())
            if n < 0:
                raise RuntimeError(f"axon_stop_nrt_profile rc={n}")

    mod = types.ModuleType("antenv.axon_hooks")
    mod.get_axon_ntff_profile_hook = lambda: _hook
    mod.set_axon_ntff_profile_hook = lambda h: None
    sys.modules["antenv.axon_hooks"] = mod
    return True


def kernel(x, pk, pv, Wqkv, Wout):
    from concourse.bass_utils import run_bass_kernel_spmd

    if "nc" not in _CACHE:
        _CACHE["nc"] = build_attn()
    nc = _CACHE["nc"]
    in_maps = make_in_maps(x, pk, pv, Wqkv, Wout)
    trace = bool(int(os.environ.get("ATTN_TRACE", "0")))
    if trace:
        trace = _install_ntff_shim()
    res = run_bass_kernel_spmd(
        nc, in_maps, core_ids=list(range(B * H)), trace=trace
    )
    _CACHE["last_results"] = res
    out = np.zeros((B, T, D), dtype=np.float32)
    for b in range(B):
        for h in range(H):
            out[b] += res.results[b * H + h]["out"]
    return out



# revision 8
# speedup vs baseline: 1.2766x; 1.2766x over previous
"""Trainium2 Bass kernel v2 for prefix-KV causal attention (nn_Attn_38757784879167).

Sharding: 8 cores <- (b, h) pairs (B=2 x H=4). Each core computes, for one
(batch, head): QKV projection and S^T-layout attention (keys on partitions,
queries on free dim). The out-projection, softmax normalization and head
summation run on the host (cheap: [4096,32]@[32,128] per head).

Perf design vs v1 (220us baseline):
  - bf16 datapath (matmul inputs), fp32 PSUM accumulation.
  - Score matmuls have contraction K=HD=32 -> 3-way ROW-TILED packs via
    tile_position=(32i,0): 3 key-chunks' score matmuls run concurrently in
    disjoint 32-row groups of the PE array. q/k are replicated across the 4
    partition groups (free: the projection matmul uses 4x-tiled weights).
  - PV matmuls (full K=128 contraction, M=33) are 2-way COL-TILED via
    tile_position=(0, (c%2)*64): consecutive chunks accumulate into
    partitions [0:33] and [64:97] of one PSUM bank concurrently.
  - exp is the true bottleneck (~17.8M elem/core, 1 elem/cyc/lane on ScalarE):
    packs are split between ScalarE (table exp, ~1.85us/3-chunk pack) and
    VectorE (Schraudolph bit-trick exp: i16 = round(s*A + B), bitcast as bf16;
    ~1.73us/pack). The shifted-causal mask folds into the VectorE op as a
    per-element fp32 bias (allowed: B, masked: B-46000 -> bitcast ~ -1e-77),
    so masking costs zero extra instructions; diagonal packs are forced to
    VectorE.
  - ~24 warmup matmuls on zeroed SBUF during the input DMA keep the PE HAM
    clock-gate warm (baseline ran its first 44us at 1.2GHz).
  - Softmax denominators ride along as a ones-column appended to v (M=33).
  - Output: the per-qtile PSUM accumulator halves [33,512]x2 are DMAed
    straight to HBM (fp32); host merges halves, normalizes, projects, sums.
"""

import math
import os

import numpy as np
import ml_dtypes

B = 2
T = 4096
D = 128
H = 4
HD = 32
PRE = 2048
CH = 128  # keys per chunk (partition dim of S^T tiles)
QT = 512  # queries per tile (free dim of S^T tiles)
NQT = T // QT
GRP = 3  # chunks per score pack / exp instruction (3 PSUM banks)
NPRE = PRE // CH  # 16 prefix chunks
NNEW = T // CH  # 32 new chunks

SCALE = 1.0 / math.sqrt(HD)
LOG2E = 1.4426950408889634
A_SCH = SCALE * LOG2E * 128.0
B_SCH = 16256.0 - 5.51
MASK_NEG = -46000.0

# packed bf16 input column offsets
WQO = 0
WKO = 128
WVO = 256
XO = 288
PKO = XO + T
PVO = PKO + PRE
NWB = PVO + NPRE * 33
NWF = 4 * QT  # maskb fp32 tensor cols

# pack-split tuning: ACT pack cost vs DVE pack cost (ns) + DVE handicap
ACT_NS = float(os.environ.get("ATTN_ACT_NS", "1850"))
DVE_NS = float(os.environ.get("ATTN_DVE_NS", "1725"))
ACT_HANDICAP = float(os.environ.get("ATTN_ACT_HC", "11400"))
DVE_HANDICAP = float(os.environ.get("ATTN_DVE_HC", "12600"))

_CACHE = {}


def build_attn():
    from contextlib import ExitStack

    import concourse.mybir as mybir
    import concourse.tile as tile
    from concourse import bacc

    f32 = mybir.dt.float32
    bf16 = mybir.dt.bfloat16
    i16 = mybir.dt.int16
    EXP = mybir.ActivationFunctionType.Exp
    MUL = mybir.AluOpType.mult
    ADD = mybir.AluOpType.add

    nc = bacc.Bacc("TRN2", target_bir_lowering=False, debug=False)

    pack_d = nc.dram_tensor("pack", [128, NWB], bf16, kind="ExternalInput")
    maskb_d = nc.dram_tensor("maskb", [128, NWF], f32, kind="ExternalInput")
    out_d = nc.dram_tensor("out", [2, NQT, 33, QT], f32, kind="ExternalOutput")

    # flattened pack list: (qtile j, c0, c1, nch)
    packs = []
    for j in range(NQT):
        nch = (PRE + QT * (j + 1)) // CH
        for p in range((nch + GRP - 1) // GRP):
            packs.append((j, GRP * p, min(GRP * p + GRP, nch), nch))

    # assignment: diagonal packs -> DVE (mask folds in); others greedy balance
    act_busy, dve_busy = ACT_HANDICAP, DVE_HANDICAP
    use_act = []
    for (j, c0, c1, nch) in packs:
        frac = (c1 - c0) / GRP
        if c1 > nch - 4:  # contains diagonal chunk
            use_act.append(False)
            dve_busy += DVE_NS * frac
        elif act_busy + ACT_NS * frac <= dve_busy + DVE_NS * frac:
            use_act.append(True)
            act_busy += ACT_NS * frac
        else:
            use_act.append(False)
            dve_busy += DVE_NS * frac

    with tile.TileContext(nc) as tc, ExitStack() as ctx:
        const = ctx.enter_context(tc.tile_pool(name="const", bufs=1))
        packed = const.tile([128, NWB], bf16, tag="pack")
        maskb = const.tile([128, NWF], f32, tag="maskb")
        qrep = const.tile([128, T], bf16, tag="qrep")
        ks = const.tile([128, T], bf16, tag="ks")
        vs = const.tile([128, NNEW * 33], bf16, tag="vs")
        wz = const.tile([128, 640], bf16, tag="wz")

        # input DMAs, ordered by first use
        nc.sync.dma_start(packed[:, 0 : XO + 2 * QT], pack_d[:, 0 : XO + 2 * QT])
        nc.sync.dma_start(packed[:, XO + 2 * QT : PKO], pack_d[:, XO + 2 * QT : PKO])
        nc.sync.dma_start(packed[:, PKO:PVO], pack_d[:, PKO:PVO])
        nc.sync.dma_start(packed[:, PVO:NWB], pack_d[:, PVO:NWB])
        nc.sync.dma_start(maskb[:], maskb_d[:])

        nc.vector.memset(wz[:], 0.0)
        vs3 = vs[:].rearrange("p (c e) -> p c e", e=33)
        nc.vector.memset(vs3[:, :, 32], 1.0)

        psS = ctx.enter_context(tc.tile_pool(name="psS", bufs=2, space="PSUM"))
        psC = ctx.enter_context(tc.tile_pool(name="psC", bufs=2, space="PSUM"))
        ptp = ctx.enter_context(tc.tile_pool(name="pt", bufs=3))
        outp = ctx.enter_context(tc.tile_pool(name="outp", bufs=2))

        # ---- HAM warmup: dummy matmuls during input DMA ----
        warm = psS.tile([128, GRP * QT], f32, tag="s")
        for _ in range(24):
            nc.tensor.matmul(warm[:, 0:QT], wz[:, 0:128], wz[:, 128:640])

        # ---- preamble: per-qtile q/k/v projections into a shared psS tile ----
        def preamble(j):
            pre = psS.tile([128, GRP * QT], f32, tag="s")
            xt = packed[:, XO + QT * j : XO + QT * (j + 1)]
            nc.tensor.matmul(pre[:, 0:QT], packed[:, WQO : WQO + 128], xt)
            nc.tensor.matmul(pre[:, QT : 2 * QT], packed[:, WKO : WKO + 128], xt)
            for i4 in range(4):
                nc.tensor.matmul(
                    pre[:, 2 * QT + 32 * i4 : 2 * QT + 32 * (i4 + 1)],
                    packed[:, XO + QT * j + CH * i4 : XO + QT * j + CH * (i4 + 1)],
                    packed[:, WVO : WVO + 32],
                )
            dst_q = qrep[:, QT * j : QT * (j + 1)]
            dst_k = ks[:, QT * j : QT * (j + 1)]
            if j % 2 == 0:
                nc.scalar.copy(dst_q, pre[:, 0:QT])
                nc.vector.tensor_copy(dst_k, pre[:, QT : 2 * QT])
            else:
                nc.vector.tensor_copy(dst_q, pre[:, 0:QT])
                nc.scalar.copy(dst_k, pre[:, QT : 2 * QT])
            nc.vector.tensor_copy(
                vs3[:, 4 * j : 4 * (j + 1), 0:32],
                pre[:, 2 * QT : 2 * QT + 128].rearrange("p (c e) -> p c e", e=32),
            )

        def k_ap(c, g):
            if c < NPRE:
                return packed[32 * g : 32 * (g + 1), PKO + CH * c : PKO + CH * (c + 1)]
            cc = c - NPRE
            return ks[32 * g : 32 * (g + 1), CH * cc : CH * (cc + 1)]

        def v_ap(c):
            if c < NPRE:
                return packed[:, PVO + 33 * c : PVO + 33 * (c + 1)]
            return vs[:, 33 * (c - NPRE) : 33 * (c - NPRE + 1)]

        psc_of = {}

        def emit_pv(j, c0, c1, nch, pt):
            psc = psc_of[j]
            for c in range(c0, c1):
                i = c - c0
                pos = (c % 2) * 64
                nc.tensor.matmul(
                    psc[pos : pos + 33, :],
                    v_ap(c),
                    pt[:, QT * i : QT * (i + 1)].bitcast(bf16),
                    start=(c < 2),
                    stop=(c >= nch - 2),
                    tile_position=(0, pos),
                    skip_group_check=True,
                )
            if c1 == nch:  # qtile complete -> stage halves in SBUF, stream out
                stag = outp.tile([128, QT], f32, tag="o", name=f"stag{j}")
                nc.scalar.copy(stag[0:33, :], psc[0:33, :])
                nc.vector.tensor_copy(stag[64:97, :], psc[64:97, :])
                nc.sync.dma_start(out_d[0, j], stag[0:33, :])
                nc.sync.dma_start(out_d[1, j], stag[64:97, :])

        preamble(0)
        preamble(1)

        prev = None
        for idx, (j, c0, c1, nch) in enumerate(packs):
            if c0 == 0:
                if j + 2 < NQT:
                    preamble(j + 2)
                psc_of[j] = psC.tile([128, QT], f32, tag="c", name=f"psc{j}")
            ps = psS.tile([128, GRP * QT], f32, tag="s")
            w = (c1 - c0) * QT
            with tc.high_priority(offset=10):
                for c in range(c0, c1):
                    i = c - c0
                    nc.tensor.matmul(
                        ps[:, QT * i : QT * (i + 1)],
                        k_ap(c, i),
                        qrep[32 * i : 32 * (i + 1), QT * j : QT * (j + 1)],
                        tile_position=(32 * i, 0),
                    )
            pt = ptp.tile([128, GRP * QT], i16, tag="pt")
            if use_act[idx]:
                nc.scalar.activation(
                    pt[:, 0:w].bitcast(bf16), ps[:, 0:w], EXP, scale=SCALE
                )
            else:
                ndiag0 = min(c1, max(c0, nch - 4)) - c0  # chunks before diag region
                if ndiag0 > 0:
                    nc.vector.tensor_scalar(
                        pt[:, 0 : QT * ndiag0],
                        ps[:, 0 : QT * ndiag0],
                        A_SCH,
                        B_SCH,
                        MUL,
                        ADD,
                    )
                if ndiag0 < c1 - c0:
                    d0 = (c0 + ndiag0) - (nch - 4)
                    nc.vector.scalar_tensor_tensor(
                        pt[:, QT * ndiag0 : w],
                        ps[:, QT * ndiag0 : w],
                        A_SCH,
                        maskb[:, QT * d0 : QT * d0 + (w - QT * ndiag0)],
                        MUL,
                        ADD,
                    )
            if prev is not None:
                emit_pv(*prev)
            prev = (j, c0, c1, nch, pt)
        emit_pv(*prev)

    nc.compile()
    return nc


def pack_inputs(x_b, pk_bh, pv_bh, wq, wk, wv):
    p = np.zeros((128, NWB), dtype=ml_dtypes.bfloat16)
    p[:, WQO : WQO + 128] = np.tile(wq, (1, 4))
    p[:, WKO : WKO + 128] = np.tile(wk, (1, 4))
    p[:, WVO : WVO + 32] = wv
    p[:, XO : XO + T] = x_b.T
    p[:, PKO : PKO + PRE] = np.tile(pk_bh.T, (4, 1))
    vimg = np.zeros((128, NPRE, 33), dtype=np.float32)
    vimg[:, :, 32] = 1.0
    vimg[:, :, 0:32] = pv_bh.reshape(NPRE, CH, HD).transpose(1, 0, 2)
    p[:, PVO:NWB] = vimg.reshape(128, -1)
    return p


def make_maskb():
    """fp32 [128, 4*512]: B_SCH where allowed (q >= k + 128d), else B_SCH+MASK_NEG."""
    m = np.full((128, NWF), B_SCH, dtype=np.float32)
    k = np.arange(128)[:, None]
    q = np.arange(QT)[None, :]
    for d in range(4):
        m[:, QT * d : QT * (d + 1)] += np.where(q >= k + CH * d, 0.0, MASK_NEG)
    return m


def make_in_maps(x, pk, pv, Wqkv, Wout):
    maskb = make_maskb()
    in_maps = []
    for b in range(B):
        for h in range(H):
            in_maps.append(
                {
                    "pack": pack_inputs(
                        np.asarray(x[b], dtype=np.float32),
                        np.asarray(pk[b, h], dtype=np.float32),
                        np.asarray(pv[b, h], dtype=np.float32),
                        np.asarray(Wqkv[:, h * HD : (h + 1) * HD], dtype=np.float32),
                        np.asarray(
                            Wqkv[:, D + h * HD : D + (h + 1) * HD], dtype=np.float32
                        ),
                        np.asarray(
                            Wqkv[:, 2 * D + h * HD : 2 * D + (h + 1) * HD],
                            dtype=np.float32,
                        ),
                    ),
                    "maskb": maskb,
                }
            )
    return in_maps


def _install_ntff_shim():
    """Provide antenv.axon_hooks (absent in this image) so trace=True works."""
    import contextlib
    import ctypes
    import sys
    import types

    try:
        from antenv.axon_hooks import get_axon_ntff_profile_hook  # noqa: F401

        return True
    except ImportError:
        pass
    so_path = "/opt/axon/libaxon_pjrt.so"
    if not os.path.exists(so_path):
        return False
    lib = ctypes.CDLL(so_path)
    if not hasattr(lib, "axon_start_nrt_profile"):
        return False
    lib.axon_start_nrt_profile.argtypes = [
        ctypes.POINTER(ctypes.c_int64),
        ctypes.c_size_t,
    ]
    lib.axon_start_nrt_profile.restype = ctypes.c_int64
    lib.axon_stop_nrt_profile.argtypes = [ctypes.c_char_p]
    lib.axon_stop_nrt_profile.restype = ctypes.c_int64

    @contextlib.contextmanager
    def _hook(output_dir, device_ids):
        import jax

        jax.devices()
        if device_ids:
            ids = (ctypes.c_int64 * len(device_ids))(*device_ids)
            rc = lib.axon_start_nrt_profile(ids, len(device_ids))
        else:
            rc = lib.axon_start_nrt_profile(None, 0)
        if rc != 0:
            raise RuntimeError(f"axon_start_nrt_profile rc={rc}")
        try:
            yield
        finally:
            n = lib.axon_stop_nrt_profile(str(output_dir).encode())
            if n < 0:
                raise RuntimeError(f"axon_stop_nrt_profile rc={n}")

    mod = types.ModuleType("antenv.axon_hooks")
    mod.get_axon_ntff_profile_hook = lambda: _hook
    mod.set_axon_ntff_profile_hook = lambda h: None
    sys.modules["antenv.axon_hooks"] = mod
    return True


def kernel(x, pk, pv, Wqkv, Wout):
    from concourse.bass_utils import run_bass_kernel_spmd

    if "nc" not in _CACHE:
        _CACHE["nc"] = build_attn()
    nc = _CACHE["nc"]
    in_maps = make_in_maps(x, pk, pv, Wqkv, Wout)
    trace = bool(int(os.environ.get("ATTN_TRACE", "0")))
    if trace:
        trace = _install_ntff_shim()
    res = run_bass_kernel_spmd(nc, in_maps, core_ids=list(range(B * H)), trace=trace)
    _CACHE["last_results"] = res
    Wout32 = np.asarray(Wout, dtype=np.float32)
    out = np.zeros((B, T, D), dtype=np.float32)
    for b in range(B):
        for h in range(H):
            r = np.asarray(res.results[b * H + h]["out"], dtype=np.float32)
            m = r[0] + r[1]  # [NQT, 33, QT]
            ctx = m[:, 0:32, :].transpose(0, 2, 1).reshape(T, HD)
            Z = m[:, 32, :].reshape(T)
            out[b] += (ctx / Z[:, None]) @ Wout32[h * HD : (h + 1) * HD, :]
    return out


# revision 13
# speedup vs baseline: 1.7051x; 1.3357x over previous
"""Trainium2 Bass kernel v2 for prefix-KV causal attention (nn_Attn_38757784879167).

Sharding: 8 cores <- (b, h) pairs (B=2 x H=4). Each core computes, for one
(batch, head): QKV projection and S^T-layout attention (keys on partitions,
queries on free dim). The out-projection, softmax normalization and head
summation run on the host (cheap: [4096,32]@[32,128] per head).

Perf design vs v1 (220us baseline):
  - bf16 datapath (matmul inputs), fp32 PSUM accumulation.
  - Score matmuls have contraction K=HD=32 -> 3-way ROW-TILED packs via
    tile_position=(32i,0): 3 key-chunks' score matmuls run concurrently in
    disjoint 32-row groups of the PE array. q/k are replicated across the 4
    partition groups (free: the projection matmul uses 4x-tiled weights).
  - PV matmuls (full K=128 contraction, M=33) are 2-way COL-TILED via
    tile_position=(0, (c%2)*64): consecutive chunks accumulate into
    partitions [0:33] and [64:97] of one PSUM bank concurrently.
  - exp is the true bottleneck (~17.8M elem/core, 1 elem/cyc/lane on ScalarE):
    packs are split between ScalarE (table exp, ~1.85us/3-chunk pack) and
    VectorE (Schraudolph bit-trick exp: i16 = round(s*A + B), bitcast as bf16;
    ~1.73us/pack). The shifted-causal mask folds into the VectorE op as a
    per-element fp32 bias (allowed: B, masked: B-46000 -> bitcast ~ -1e-77),
    so masking costs zero extra instructions; diagonal packs are forced to
    VectorE.
  - ~24 warmup matmuls on zeroed SBUF during the input DMA keep the PE HAM
    clock-gate warm (baseline ran its first 44us at 1.2GHz).
  - Softmax denominators ride along as a ones-column appended to v (M=33).
  - Output: the per-qtile PSUM accumulator halves [33,512]x2 are DMAed
    straight to HBM (fp32); host merges halves, normalizes, projects, sums.
"""

import math
import os

import numpy as np
import ml_dtypes

B = 2
T = 4096
D = 128
H = 4
HD = 32
PRE = 2048
CH = 128  # keys per chunk (partition dim of S^T tiles)
QT = 512  # queries per tile (free dim of S^T tiles)
NQT = T // QT
GRP = 3  # chunks per score pack / exp instruction (3 PSUM banks)
NPRE = PRE // CH  # 16 prefix chunks
NNEW = T // CH  # 32 new chunks

SCALE = 1.0 / math.sqrt(HD)
LOG2E = 1.4426950408889634
A_SCH = SCALE * LOG2E * 128.0
B_SCH = 16256.0 - 5.51
MASK_NEG = -46000.0

# packed bf16 input column offsets
WQO = 0
WKO = 128
WVO = 256
XO = 288
PKO = XO + T
PVO = PKO + PRE
NWB = PVO + NPRE * 33
NWF = 4 * QT  # maskb fp32 tensor cols

# pack-split tuning: ACT pack cost vs DVE pack cost (ns) + DVE handicap
ACT_NS = float(os.environ.get("ATTN_ACT_NS", "1570"))
DVE_NS = float(os.environ.get("ATTN_DVE_NS", "2150"))
ACT_HANDICAP = float(os.environ.get("ATTN_ACT_HC", "15000"))
DVE_HANDICAP = float(os.environ.get("ATTN_DVE_HC", "13500"))

_CACHE = {}


def build_attn():
    from contextlib import ExitStack

    import concourse.mybir as mybir
    import concourse.tile as tile
    from concourse import bacc

    f32 = mybir.dt.float32
    bf16 = mybir.dt.bfloat16
    i16 = mybir.dt.int16
    EXP = mybir.ActivationFunctionType.Exp
    MUL = mybir.AluOpType.mult
    ADD = mybir.AluOpType.add

    nc = bacc.Bacc("TRN2", target_bir_lowering=False, debug=False)

    pack_d = nc.dram_tensor("pack", [128, NWB], bf16, kind="ExternalInput")
    maskb_d = nc.dram_tensor("maskb", [128, NWF], f32, kind="ExternalInput")
    out_d = nc.dram_tensor("out", [2, NQT, 33, QT], f32, kind="ExternalOutput")

    # flattened pack list: (qtile j, c0, c1, nch)
    packs = []
    for j in range(NQT):
        nch = (PRE + QT * (j + 1)) // CH
        for p in range((nch + GRP - 1) // GRP):
            packs.append((j, GRP * p, min(GRP * p + GRP, nch), nch))

    # assignment: diagonal packs -> DVE (mask folds in); the pack just before a
    # diagonal run -> ACT (keeps the engines interleaved); others greedy balance
    act_busy, dve_busy = ACT_HANDICAP, DVE_HANDICAP
    use_act = []
    for pi, (j, c0, c1, nch) in enumerate(packs):
        frac = (c1 - c0) / GRP
        nxt = packs[pi + 1] if pi + 1 < len(packs) else None
        if c1 > nch - 4:  # contains diagonal chunk
            use_act.append(False)
            dve_busy += DVE_NS * frac
        elif nxt is not None and nxt[0] == j and nxt[2] > nxt[3] - 4:
            use_act.append(True)  # next pack is diagonal (DVE): force ACT here
            act_busy += ACT_NS * frac
        elif act_busy + ACT_NS * frac <= dve_busy + DVE_NS * frac:
            use_act.append(True)
            act_busy += ACT_NS * frac
        else:
            use_act.append(False)
            dve_busy += DVE_NS * frac

    with tile.TileContext(nc) as tc, ExitStack() as ctx:
        const = ctx.enter_context(tc.tile_pool(name="const", bufs=1))
        packed = const.tile([128, NWB], bf16, tag="pack")
        maskb = const.tile([128, NWF], f32, tag="maskb")
        qrep = const.tile([128, T], bf16, tag="qrep")
        ks = const.tile([128, T], bf16, tag="ks")
        vs = const.tile([128, NNEW * 33], bf16, tag="vs")
        wz = const.tile([128, 640], bf16, tag="wz")

        # input DMAs, ordered by first use
        nc.sync.dma_start(packed[:, 0 : XO + 2 * QT], pack_d[:, 0 : XO + 2 * QT])
        nc.sync.dma_start(packed[:, XO + 2 * QT : PKO], pack_d[:, XO + 2 * QT : PKO])
        nc.sync.dma_start(packed[:, PKO:PVO], pack_d[:, PKO:PVO])
        nc.sync.dma_start(packed[:, PVO:NWB], pack_d[:, PVO:NWB])
        nc.sync.dma_start(maskb[:], maskb_d[:])

        nc.vector.memset(wz[:], 0.0)
        vs3 = vs[:].rearrange("p (c e) -> p c e", e=33)
        nc.vector.memset(vs3[:, :, 32], 1.0)

        psS = ctx.enter_context(tc.tile_pool(name="psS", bufs=2, space="PSUM"))
        psC = ctx.enter_context(tc.tile_pool(name="psC", bufs=2, space="PSUM"))
        ptp = ctx.enter_context(tc.tile_pool(name="pt", bufs=4))
        outp = ctx.enter_context(tc.tile_pool(name="outp", bufs=2))

        # ---- HAM warmup: dummy matmuls during input DMA ----
        warm = psS.tile([128, GRP * QT], f32, tag="s")
        for _ in range(10):
            nc.tensor.matmul(warm[:, 0:QT], wz[:, 0:128], wz[:, 128:640])

        # ---- preamble: per-qtile q/k/v projections into a shared psS tile ----
        def preamble(j):
            pre = psS.tile([128, GRP * QT], f32, tag="s")
            xt = packed[:, XO + QT * j : XO + QT * (j + 1)]
            nc.tensor.matmul(pre[:, 0:QT], packed[:, WQO : WQO + 128], xt)
            nc.tensor.matmul(pre[:, QT : 2 * QT], packed[:, WKO : WKO + 128], xt)
            for i4 in range(4):
                nc.tensor.matmul(
                    pre[:, 2 * QT + 32 * i4 : 2 * QT + 32 * (i4 + 1)],
                    packed[:, XO + QT * j + CH * i4 : XO + QT * j + CH * (i4 + 1)],
                    packed[:, WVO : WVO + 32],
                )
            dst_q = qrep[:, QT * j : QT * (j + 1)]
            dst_k = ks[:, QT * j : QT * (j + 1)]
            if j % 2 == 0:
                nc.scalar.copy(dst_q, pre[:, 0:QT])
                nc.vector.tensor_copy(dst_k, pre[:, QT : 2 * QT])
            else:
                nc.vector.tensor_copy(dst_q, pre[:, 0:QT])
                nc.scalar.copy(dst_k, pre[:, QT : 2 * QT])
            nc.vector.tensor_copy(
                vs3[:, 4 * j : 4 * (j + 1), 0:32],
                pre[:, 2 * QT : 2 * QT + 128].rearrange("p (c e) -> p c e", e=32),
            )

        def k_ap(c, g):
            if c < NPRE:
                return packed[32 * g : 32 * (g + 1), PKO + CH * c : PKO + CH * (c + 1)]
            cc = c - NPRE
            return ks[32 * g : 32 * (g + 1), CH * cc : CH * (cc + 1)]

        def v_ap(c):
            if c < NPRE:
                return packed[:, PVO + 33 * c : PVO + 33 * (c + 1)]
            return vs[:, 33 * (c - NPRE) : 33 * (c - NPRE + 1)]

        psc_of = {}

        def emit_pv(j, c0, c1, nch, pt):
            psc = psc_of[j]
            for c in range(c0, c1):
                i = c - c0
                pos = (c % 2) * 64
                nc.tensor.matmul(
                    psc[pos : pos + 33, :],
                    v_ap(c),
                    pt[:, QT * i : QT * (i + 1)].bitcast(bf16),
                    start=(c < 2),
                    stop=(c >= nch - 2),
                    tile_position=(0, pos),
                    skip_group_check=True,
                )
            if c1 == nch:  # qtile complete -> stage halves in SBUF, stream out
                stag = outp.tile([128, QT], f32, tag="o", name=f"stag{j}")
                nc.scalar.copy(stag[0:33, :], psc[0:33, :])
                nc.vector.tensor_copy(stag[64:97, :], psc[64:97, :])
                nc.sync.dma_start(out_d[0, j], stag[0:33, :])
                nc.sync.dma_start(out_d[1, j], stag[64:97, :])

        for j in range(NQT):
            preamble(j)

        prev = None
        for idx, (j, c0, c1, nch) in enumerate(packs):
            if c0 == 0:
                psc_of[j] = psC.tile([128, QT], f32, tag="c", name=f"psc{j}")
            ps = psS.tile([128, GRP * QT], f32, tag="s")
            w = (c1 - c0) * QT
            with tc.high_priority(offset=10):
                for c in range(c0, c1):
                    i = c - c0
                    nc.tensor.matmul(
                        ps[:, QT * i : QT * (i + 1)],
                        k_ap(c, i),
                        qrep[32 * i : 32 * (i + 1), QT * j : QT * (j + 1)],
                        tile_position=(32 * i, 0),
                    )
            pt = ptp.tile([128, GRP * QT], i16, tag="pt")
            if use_act[idx]:
                nc.scalar.activation(
                    pt[:, 0:w].bitcast(bf16), ps[:, 0:w], EXP, scale=SCALE
                )
            else:
                ndiag0 = min(c1, max(c0, nch - 4)) - c0  # chunks before diag region
                if ndiag0 > 0:
                    nc.vector.tensor_scalar(
                        pt[:, 0 : QT * ndiag0],
                        ps[:, 0 : QT * ndiag0],
                        A_SCH,
                        B_SCH,
                        MUL,
                        ADD,
                    )
                if ndiag0 < c1 - c0:
                    d0 = (c0 + ndiag0) - (nch - 4)
                    nc.vector.scalar_tensor_tensor(
                        pt[:, QT * ndiag0 : w],
                        ps[:, QT * ndiag0 : w],
                        A_SCH,
                        maskb[:, QT * d0 : QT * d0 + (w - QT * ndiag0)],
                        MUL,
                        ADD,
                    )
            if prev is not None:
                emit_pv(*prev)
            prev = (j, c0, c1, nch, pt)
        emit_pv(*prev)

    nc.compile()
    return nc


def pack_inputs(x_b, pk_bh, pv_bh, wq, wk, wv):
    p = np.zeros((128, NWB), dtype=ml_dtypes.bfloat16)
    p[:, WQO : WQO + 128] = np.tile(wq, (1, 4))
    p[:, WKO : WKO + 128] = np.tile(wk, (1, 4))
    p[:, WVO : WVO + 32] = wv
    p[:, XO : XO + T] = x_b.T
    p[:, PKO : PKO + PRE] = np.tile(pk_bh.T, (4, 1))
    vimg = np.zeros((128, NPRE, 33), dtype=np.float32)
    vimg[:, :, 32] = 1.0
    vimg[:, :, 0:32] = pv_bh.reshape(NPRE, CH, HD).transpose(1, 0, 2)
    p[:, PVO:NWB] = vimg.reshape(128, -1)
    return p


def make_maskb():
    """fp32 [128, 4*512]: B_SCH where allowed (q >= k + 128d), else B_SCH+MASK_NEG."""
    m = np.full((128, NWF), B_SCH, dtype=np.float32)
    k = np.arange(128)[:, None]
    q = np.arange(QT)[None, :]
    for d in range(4):
        m[:, QT * d : QT * (d + 1)] += np.where(q >= k + CH * d, 0.0, MASK_NEG)
    return m


def make_in_maps(x, pk, pv, Wqkv, Wout):
    maskb = make_maskb()
    in_maps = []
    for b in range(B):
        for h in range(H):
            in_maps.append(
                {
                    "pack": pack_inputs(
                        np.asarray(x[b], dtype=np.float32),
                        np.asarray(pk[b, h], dtype=np.float32),
                        np.asarray(pv[b, h], dtype=np.float32),
                        np.asarray(Wqkv[:, h * HD : (h + 1) * HD], dtype=np.float32),
                        np.asarray(
                            Wqkv[:, D + h * HD : D + (h + 1) * HD], dtype=np.float32
                        ),
                        np.asarray(
                            Wqkv[:, 2 * D + h * HD : 2 * D + (h + 1) * HD],
                            dtype=np.float32,
                        ),
                    ),
                    "maskb": maskb,
                }
            )
    return in_maps


def _install_ntff_shim():
    """Provide antenv.axon_hooks (absent in this image) so trace=True works."""
    import contextlib
    import ctypes
    import sys
    import types

    try:
        from antenv.axon_hooks import get_axon_ntff_profile_hook  # noqa: F401

        return True
    except ImportError:
        pass
    so_path = "/opt/axon/libaxon_pjrt.so"
    if not os.path.exists(so_path):
        return False
    lib = ctypes.CDLL(so_path)
    if not hasattr(lib, "axon_start_nrt_profile"):
        return False
    lib.axon_start_nrt_profile.argtypes = [
        ctypes.POINTER(ctypes.c_int64),
        ctypes.c_size_t,
    ]
    lib.axon_start_nrt_profile.restype = ctypes.c_int64
    lib.axon_stop_nrt_profile.argtypes = [ctypes.c_char_p]
    lib.axon_stop_nrt_profile.restype = ctypes.c_int64

    @contextlib.contextmanager
    def _hook(output_dir, device_ids):
        import jax

        jax.devices()
        if device_ids:
            ids = (ctypes.c_int64 * len(device_ids))(*device_ids)
            rc = lib.axon_start_nrt_profile(ids, len(device_ids))
        else:
            rc = lib.axon_start_nrt_profile(None, 0)
        if rc != 0:
            raise RuntimeError(f"axon_start_nrt_profile rc={rc}")
        try:
            yield
        finally:
            n = lib.axon_stop_nrt_profile(str(output_dir).encode())
            if n < 0:
                raise RuntimeError(f"axon_stop_nrt_profile rc={n}")

    mod = types.ModuleType("antenv.axon_hooks")
    mod.get_axon_ntff_profile_hook = lambda: _hook
    mod.set_axon_ntff_profile_hook = lambda h: None
    sys.modules["antenv.axon_hooks"] = mod
    return True


def kernel(x, pk, pv, Wqkv, Wout):
    from concourse.bass_utils import run_bass_kernel_spmd

    if "nc" not in _CACHE:
        _CACHE["nc"] = build_attn()
    nc = _CACHE["nc"]
    in_maps = make_in_maps(x, pk, pv, Wqkv, Wout)
    trace = bool(int(os.environ.get("ATTN_TRACE", "0")))
    if trace:
        trace = _install_ntff_shim()
    res = run_bass_kernel_spmd(nc, in_maps, core_ids=list(range(B * H)), trace=trace)
    _CACHE["last_results"] = res
    Wout32 = np.asarray(Wout, dtype=np.float32)
    out = np.zeros((B, T, D), dtype=np.float32)
    for b in range(B):
        for h in range(H):
            r = np.asarray(res.results[b * H + h]["out"], dtype=np.float32)
            m = r[0] + r[1]  # [NQT, 33, QT]
            ctx = m[:, 0:32, :].transpose(0, 2, 1).reshape(T, HD)
            Z = m[:, 32, :].reshape(T)
            out[b] += (ctx / Z[:, None]) @ Wout32[h * HD : (h + 1) * HD, :]
    return out
